# revision 25
# baseline (speedup 1.0000x reference)
"""Trainium2 Bass kernel for causal average pooling (downsampling).

Reference op: out[b, i, d] = mean(x[b, :(i+1)*4, d]) over the time axis,
for x of shape (8, 8192, 512) f32 -> out (8, 2048, 512) f32.

Strategy
--------
Data-parallel over batch: one batch per NeuronCore (8 cores), no
cross-core communication.

Per core the math is, for each channel d independently, a prefix sum
over time sampled every SF=4 steps, scaled by 1/(4(i+1)).  We lay the
data out as [channel partitions, time free-dim] (the host pre-transposes
each batch, which is pure layout) and use the hardware prefix scan
`tensor_tensor_scan` on the vector engine:

    state = (data0[t] + state) + data1[t]

Feeding data0 = x[:, 0::2] and data1 = x[:, 1::2] gives the cumulative
sum over PAIRS: cs2[:, j] = sum(x[:, :2j+2]).  Output i of the reference
needs sum(x[:, :4i+4]) = cs2[:, 2i+1]: a strided gather of the odd
columns times a 1/(4(i+1)) table (an 8 KB host row, partition-broadcast
on GpSimd so it costs no DMA-fabric bandwidth).

Each 128-channel tile's time axis is cut into pieces which are scanned
INDEPENDENTLY (initial=0.0 — chaining through an AP initial measured
~2.3 us slower per scan).  A piece's missing carry (total of the earlier
pieces, maintained as a [128,1] running column) is folded into its
output op for free with scalar_tensor_tensor:
    out = (cs_local + carry) * recip.
The kernel is DMA-fabric-bound (~430 GB/s shared by loads+stores), so
the last tile is tapered into small pieces to shrink the serial tail
(last-load receipt -> scan -> out -> store -> receipt).

Pipeline per core (xT [512 chan, 8192 time], 4 channel tiles):
  SP ring:   x piece loads (2 MiB mid-stream, tapered at the end)
  ACT ring:  recip row load (8 KB), then per-piece output stores
  GpSimd:    recip partition-broadcast (once)
  DVE:       per piece: scan + gather*recip (TT / STT), carry columns

Written in raw Bass (not Tile): the walrus build in this container
enforces at most ONE semaphore wait per hardware instruction, so all
cross-engine waits are standalone wait_ge ops.  Each load gets its own
semaphore because completions of back-to-back DMAs on one HWDGE ring
are unordered.
"""

import sys

if "/opt/trn_rl_repo" not in sys.path:
    sys.path.insert(0, "/opt/trn_rl_repo")

import numpy as np

import concourse.bass as bass
import concourse.mybir as mybir
from concourse.bass_utils import run_bass_kernel_spmd

P = 128           # SBUF partitions
SF = 4            # pooling factor
B, L, D = 8, 8192, 512
N_CORES = 8


def _pieces(n_ct, length):
    """Per-tile piece boundaries in x columns. Mid-stream pieces are half a
    tile (2 MiB); the first tile ramps up in small pieces (first scan starts
    after 0.5 MiB instead of 2 MiB) and the final tile tapers down so the
    serial tail after the last load is short."""
    halves = [(0, length // 2), (length // 2, length)]
    if length < 4096:
        return [halves] * n_ct
    ramp = [
        (0, length // 4),
        (length // 4, length // 2),
        (length // 2, length),
    ]
    taper = [
        (0, length // 2),
        (length // 2, 3 * length // 4),
        (3 * length // 4, 7 * length // 8),
        (7 * length // 8, length),
    ]
    return [ramp] + [halves] * (n_ct - 2) + [taper]


def build_bass(d=D, length=L):
    half = length // 2          # scan steps per tile (pairs)
    out_len = length // SF
    n_ct = d // P
    assert d % P == 0 and length % (2 * SF * 8) == 0

    nc = bass.Bass()
    xT = nc.dram_tensor("xT", [d, length], mybir.dt.float32, kind="ExternalInput")
    recip = nc.dram_tensor(
        "recip", [1, out_len], mybir.dt.float32, kind="ExternalInput"
    )
    outT = nc.dram_tensor(
        "outT", [d, out_len], mybir.dt.float32, kind="ExternalOutput"
    )

    pieces = _pieces(n_ct, length)
    n_loads = sum(len(p) for p in pieces)

    # DVE op index bookkeeping (s_cmp is incremented by every DVE op).
    cmp_val = 0
    scan_val = [[None] * len(pieces[ct]) for ct in range(n_ct)]  # scan done
    out_val = [[None] * len(pieces[ct]) for ct in range(n_ct)]   # out op done

    with (
        nc.sbuf_tensor([P, length], mybir.dt.float32) as xt0,
        nc.sbuf_tensor([P, length], mybir.dt.float32) as xt1,
        nc.sbuf_tensor([P, half], mybir.dt.float32) as cs0,
        nc.sbuf_tensor([P, half], mybir.dt.float32) as cs1,
        nc.sbuf_tensor([1, out_len], mybir.dt.float32) as rrow,
        nc.sbuf_tensor([1, P], mybir.dt.float32) as ones,
        nc.psum_tensor([P, out_len], mybir.dt.float32) as rps,
        nc.sbuf_tensor([P, out_len], mybir.dt.float32) as rt,
        nc.sbuf_tensor([P, n_ct], mybir.dt.float32) as runc,
        nc.sbuf_tensor([P, n_ct, out_len], mybir.dt.float32) as ot,
        nc.semaphore("s_rrow") as s_rrow,
        nc.semaphore("s_ones") as s_ones,
        nc.semaphore("s_ps") as s_ps,
        nc.semaphore("s_rt") as s_rt,
        nc.semaphore("s_cmp") as s_cmp,
        nc.semaphore("s_out") as s_out,
        nc.Block() as block,
    ):
        n_banks = (out_len + 511) // 512
        bank_cols = min(512, out_len)
        s_xs = [nc.alloc_semaphore(f"s_x{i}") for i in range(n_loads)]
        xts = [xt0, xt1]
        css = [cs0, cs1]

        # ---- plan the DVE op order so cross-engine wait values are known ---
        # per tile: [scan p, (run-col update?), out p] for each piece p
        for ct in range(n_ct):
            for p in range(len(pieces[ct])):
                cmp_val += 1                     # scan
                scan_val[ct][p] = cmp_val
                if p >= 2:
                    cmp_val += 1                 # running-carry column update
                cmp_val += 1                     # out op
                out_val[ct][p] = cmp_val

        @block.sync
        def _(sync):
            # 8 KB recip row first (negligible bytes, needed by the PE
            # broadcast early), then x loads, all on the SP HWDGE ring.
            sync.dma_start(out=rrow[:, :], in_=recip[:, :]).then_inc(s_rrow, 16)
            li = 0
            for ct in range(n_ct):
                for p, (xs, xe) in enumerate(pieces[ct]):
                    if ct >= 2:
                        # buffer WAR: last scan of tile ct-2 whose region
                        # overlaps this piece must be done with the buffer.
                        last = max(
                            pp for pp, (ps, pe) in enumerate(pieces[ct - 2])
                            if ps < xe and pe > xs
                        )
                        sync.wait_ge(s_cmp, scan_val[ct - 2][last])
                    sync.dma_start(
                        out=xts[ct % 2][:, xs:xe],
                        in_=xT[ct * P:(ct + 1) * P, xs:xe],
                    ).then_inc(s_xs[li], 16)
                    li += 1

        @block.gpsimd
        def _(gpsimd):
            nc.gpsimd.memset(ones[:, :], 1.0).then_inc(s_ones, 1)

        @block.tensor
        def _(tensor):
            # Broadcast the 8 KB recip row to all 128 partitions on the
            # (otherwise idle) PE: ones[1,128].T @ rrow[1,bank] replicates the
            # row into PSUM, so the table never crosses the DMA fabric at
            # full size.  One matmul per PSUM bank (N<=512).
            tensor.wait_ge(s_rrow, 16)
            tensor.wait_ge(s_ones, 1)
            for k in range(n_banks):
                nc.tensor.matmul(
                    rps[:, k * bank_cols:(k + 1) * bank_cols],
                    ones[:, :],
                    rrow[:, k * bank_cols:(k + 1) * bank_cols],
                    start=True,
                    stop=True,
                ).then_inc(s_ps, 1)

        @block.vector
        def _(vector):
            li = 0
            cval = 0
            for ct in range(n_ct):
                cs = css[ct % 2][:, :]
                xtile = xts[ct % 2]
                for p, (xs, xe) in enumerate(pieces[ct]):
                    c0, c1 = xs // 2, xe // 2    # cs (pair) columns
                    o0, o1 = xs // 4, xe // 4    # output columns
                    vector.wait_ge(s_xs[li], 16)
                    li += 1
                    if ct >= 2:
                        # cs WAW vs tile ct-2's final out; trivially satisfied
                        # by DVE program order, stated for the race checker.
                        vector.wait_ge(s_cmp, out_val[ct - 2][-1])
                    xv = xtile[:, xs:xe].rearrange("p (t two) -> p t two", two=2)
                    nc.vector.tensor_tensor_scan(
                        cs[:, c0:c1],
                        xv[:, :, 0],
                        xv[:, :, 1],
                        0.0,
                        mybir.AluOpType.add,
                        mybir.AluOpType.add,
                    ).then_inc(s_cmp, 1)
                    cval += 1
                    assert cval == scan_val[ct][p]
                    # scan -> out RAW on the same engine; for the checker.
                    vector.wait_ge(s_cmp, cval)
                    if p >= 2:
                        # carry column: total of pieces 0..p-1.
                        prev_end = pieces[ct][p - 1][1] // 2
                        if p == 2:
                            first_end = pieces[ct][0][1] // 2
                            nc.vector.tensor_add(
                                runc[:, ct:ct + 1],
                                cs[:, first_end - 1:first_end],
                                cs[:, prev_end - 1:prev_end],
                            ).then_inc(s_cmp, 1)
                        else:
                            nc.vector.tensor_add(
                                runc[:, ct:ct + 1],
                                runc[:, ct:ct + 1],
                                cs[:, prev_end - 1:prev_end],
                            ).then_inc(s_cmp, 1)
                        cval += 1
                        vector.wait_ge(s_cmp, cval)
                    if ct == 0 and p == 0:
                        # The recip table is only needed from the first OUT
                        # op on — never gate the scans on it.
                        vector.wait_ge(s_rt, n_banks)
                    csv = cs[:, c0:c1].rearrange("p (t two) -> p t two", two=2)
                    o_ap = ot[:, ct, o0:o1]
                    r_ap = rt[:, o0:o1]
                    if p == 0:
                        nc.vector.tensor_mul(
                            o_ap, csv[:, :, 1], r_ap
                        ).then_inc(s_cmp, 1)
                    elif p == 1:
                        # carry is just piece 0's total column.
                        nc.vector.scalar_tensor_tensor(
                            o_ap, csv[:, :, 1], cs[:, c0 - 1:c0], r_ap,
                            mybir.AluOpType.add, mybir.AluOpType.mult,
                        ).then_inc(s_cmp, 1)
                    else:
                        nc.vector.scalar_tensor_tensor(
                            o_ap, csv[:, :, 1], runc[:, ct:ct + 1], r_ap,
                            mybir.AluOpType.add, mybir.AluOpType.mult,
                        ).then_inc(s_cmp, 1)
                    cval += 1
                    assert cval == out_val[ct][p]

        @block.scalar
        def _(scalar):
            # Output stores on the ACT HWDGE ring; the PSUM->SBUF copies of
            # the broadcast recip table run on the idle ACT ALU.
            for k in range(n_banks):
                scalar.wait_ge(s_ps, k + 1)
                nc.scalar.copy(
                    rt[:, k * bank_cols:(k + 1) * bank_cols],
                    rps[:, k * bank_cols:(k + 1) * bank_cols],
                ).then_inc(s_rt, 1)
            n_stores = 0
            for ct in range(n_ct):
                for p, (xs, xe) in enumerate(pieces[ct]):
                    o0, o1 = xs // 4, xe // 4
                    scalar.wait_ge(s_cmp, out_val[ct][p])
                    scalar.dma_start(
                        out=outT[ct * P:(ct + 1) * P, o0:o1],
                        in_=ot[:, ct, o0:o1],
                    ).then_inc(s_out, 16)
                    n_stores += 1
            # Outputs must be in HBM before the kernel exits.
            scalar.wait_ge(s_out, 16 * n_stores)

    return nc


def _recip_row(out_len):
    r = 1.0 / (SF * np.arange(1, out_len + 1, dtype=np.float64))
    return r.astype(np.float32).reshape(1, out_len)


def kernel(x: np.ndarray) -> np.ndarray:
    b, length, d = x.shape
    out_len = length // SF
    # One batch per core, channels on partitions: host-side transpose is
    # pure layout so every DMA in the kernel is contiguous.
    xT = np.ascontiguousarray(np.swapaxes(np.asarray(x, dtype=np.float32), 1, 2))
    recip = _recip_row(out_len)
    in_maps = [{"xT": xT[i], "recip": recip} for i in range(b)]
    nc = build_bass(d=d, length=length)
    res = run_bass_kernel_spmd(nc, in_maps, core_ids=list(range(b)))
    outT = np.stack([res.results[i]["outT"] for i in range(b)])
    return np.ascontiguousarray(np.swapaxes(outT, 1, 2))


# revision 26
# speedup vs baseline: 1.0555x; 1.0555x over previous
"""Trainium2 Bass kernel for causal average pooling (downsampling).

Reference op: out[b, i, d] = mean(x[b, :(i+1)*4, d]) over the time axis,
for x of shape (8, 8192, 512) f32 -> out (8, 2048, 512) f32.

Strategy
--------
Data-parallel over batch: one batch per NeuronCore (8 cores), no
cross-core communication.

Per core the math is, for each channel d independently, a prefix sum
over time sampled every SF=4 steps, scaled by 1/(4(i+1)).  We lay the
data out as [channel partitions, time free-dim] (the host pre-transposes
each batch, which is pure layout) and use the hardware prefix scan
`tensor_tensor_scan` on the vector engine:

    state = (data0[t] + state) + data1[t]

Feeding data0 = x[:, 0::2] and data1 = x[:, 1::2] gives the cumulative
sum over PAIRS: cs2[:, j] = sum(x[:, :2j+2]).  Output i of the reference
needs sum(x[:, :4i+4]) = cs2[:, 2i+1]: a strided gather of the odd
columns times a 1/(4(i+1)) table (an 8 KB host row, partition-broadcast
on GpSimd so it costs no DMA-fabric bandwidth).

Each 128-channel tile's time axis is cut into pieces which are scanned
INDEPENDENTLY (initial=0.0 — chaining through an AP initial measured
~2.3 us slower per scan).  A piece's missing carry (total of the earlier
pieces, maintained as a [128,1] running column) is folded into its
output op for free with scalar_tensor_tensor:
    out = (cs_local + carry) * recip.
The kernel is DMA-fabric-bound (~430 GB/s shared by loads+stores), so
the last tile is tapered into small pieces to shrink the serial tail
(last-load receipt -> scan -> out -> store -> receipt).

Pipeline per core (xT [512 chan, 8192 time], 4 channel tiles):
  SP ring:   x piece loads (2 MiB mid-stream, tapered at the end)
  ACT ring:  recip row load (8 KB), then per-piece output stores
  GpSimd:    recip partition-broadcast (once)
  DVE:       per piece: scan + gather*recip (TT / STT), carry columns

Written in raw Bass (not Tile): the walrus build in this container
enforces at most ONE semaphore wait per hardware instruction, so all
cross-engine waits are standalone wait_ge ops.  Each load gets its own
semaphore because completions of back-to-back DMAs on one HWDGE ring
are unordered.
"""

import sys

if "/opt/trn_rl_repo" not in sys.path:
    sys.path.insert(0, "/opt/trn_rl_repo")

import numpy as np

import concourse.bass as bass
import concourse.mybir as mybir
from concourse.bass_utils import run_bass_kernel_spmd

P = 128           # SBUF partitions
SF = 4            # pooling factor
B, L, D = 8, 8192, 512
N_CORES = 8


def _pieces(n_ct, length):
    """Per-tile piece boundaries in x columns. Mid-stream pieces are half a
    tile (2 MiB); the first tile ramps up in small pieces (first scan starts
    after 0.5 MiB instead of 2 MiB) and the final tile tapers down so the
    serial tail after the last load is short."""
    halves = [(0, length // 2), (length // 2, length)]
    if length < 4096:
        return [halves] * n_ct
    ramp = halves
    taper = [
        (0, length // 2),
        (length // 2, 3 * length // 4),
        (3 * length // 4, 7 * length // 8),
        (7 * length // 8, length),
    ]
    return [ramp] + [halves] * (n_ct - 2) + [taper]


def build_bass(d=D, length=L):
    half = length // 2          # scan steps per tile (pairs)
    out_len = length // SF
    n_ct = d // P
    assert d % P == 0 and length % (2 * SF * 8) == 0

    nc = bass.Bass()
    xT = nc.dram_tensor("xT", [d, length], mybir.dt.float32, kind="ExternalInput")
    recip = nc.dram_tensor(
        "recip", [1, out_len], mybir.dt.float32, kind="ExternalInput"
    )
    outT = nc.dram_tensor(
        "outT", [d, out_len], mybir.dt.float32, kind="ExternalOutput"
    )

    pieces = _pieces(n_ct, length)
    n_loads = sum(len(p) for p in pieces)

    # DVE op index bookkeeping (s_cmp is incremented by every DVE op).
    cmp_val = 0
    scan_val = [[None] * len(pieces[ct]) for ct in range(n_ct)]  # scan done
    out_val = [[None] * len(pieces[ct]) for ct in range(n_ct)]   # out op done

    with (
        nc.sbuf_tensor([P, length], mybir.dt.float32) as xt0,
        nc.sbuf_tensor([P, length], mybir.dt.float32) as xt1,
        nc.sbuf_tensor([P, half], mybir.dt.float32) as cs0,
        nc.sbuf_tensor([P, half], mybir.dt.float32) as cs1,
        nc.sbuf_tensor([1, out_len], mybir.dt.float32) as rrow,
        nc.sbuf_tensor([1, P], mybir.dt.float32) as ones,
        nc.psum_tensor([P, out_len], mybir.dt.float32) as rps,
        nc.sbuf_tensor([P, out_len], mybir.dt.float32) as rt,
        nc.sbuf_tensor([P, n_ct], mybir.dt.float32) as runc,
        nc.sbuf_tensor([P, n_ct, out_len], mybir.dt.float32) as ot,
        nc.semaphore("s_rrow") as s_rrow,
        nc.semaphore("s_ones") as s_ones,
        nc.semaphore("s_ps") as s_ps,
        nc.semaphore("s_rt") as s_rt,
        nc.semaphore("s_cmp") as s_cmp,
        nc.semaphore("s_out") as s_out,
        nc.Block() as block,
    ):
        n_banks = (out_len + 511) // 512
        bank_cols = min(512, out_len)
        s_xs = [nc.alloc_semaphore(f"s_x{i}") for i in range(n_loads)]
        xts = [xt0, xt1]
        css = [cs0, cs1]

        # ---- plan the DVE op order so cross-engine wait values are known ---
        # per tile: [scan p, (run-col update?), out p] for each piece p
        for ct in range(n_ct):
            for p in range(len(pieces[ct])):
                cmp_val += 1                     # scan
                scan_val[ct][p] = cmp_val
                if p >= 2:
                    cmp_val += 1                 # running-carry column update
                cmp_val += 1                     # out op
                out_val[ct][p] = cmp_val

        @block.sync
        def _(sync):
            # 8 KB recip row first (negligible bytes, needed by the PE
            # broadcast early), then x loads, all on the SP HWDGE ring.
            sync.dma_start(out=rrow[:, :], in_=recip[:, :]).then_inc(s_rrow, 16)
            li = 0
            for ct in range(n_ct):
                for p, (xs, xe) in enumerate(pieces[ct]):
                    if ct >= 2:
                        # buffer WAR: last scan of tile ct-2 whose region
                        # overlaps this piece must be done with the buffer.
                        last = max(
                            pp for pp, (ps, pe) in enumerate(pieces[ct - 2])
                            if ps < xe and pe > xs
                        )
                        sync.wait_ge(s_cmp, scan_val[ct - 2][last])
                    sync.dma_start(
                        out=xts[ct % 2][:, xs:xe],
                        in_=xT[ct * P:(ct + 1) * P, xs:xe],
                    ).then_inc(s_xs[li], 16)
                    li += 1

        @block.gpsimd
        def _(gpsimd):
            nc.gpsimd.memset(ones[:, :], 1.0).then_inc(s_ones, 1)

        @block.tensor
        def _(tensor):
            # Broadcast the 8 KB recip row to all 128 partitions on the
            # (otherwise idle) PE: ones[1,128].T @ rrow[1,bank] replicates the
            # row into PSUM, so the table never crosses the DMA fabric at
            # full size.  One matmul per PSUM bank (N<=512).
            tensor.wait_ge(s_rrow, 16)
            tensor.wait_ge(s_ones, 1)
            for k in range(n_banks):
                nc.tensor.matmul(
                    rps[:, k * bank_cols:(k + 1) * bank_cols],
                    ones[:, :],
                    rrow[:, k * bank_cols:(k + 1) * bank_cols],
                    start=True,
                    stop=True,
                ).then_inc(s_ps, 1)

        @block.vector
        def _(vector):
            li = 0
            cval = 0
            for ct in range(n_ct):
                cs = css[ct % 2][:, :]
                xtile = xts[ct % 2]
                for p, (xs, xe) in enumerate(pieces[ct]):
                    c0, c1 = xs // 2, xe // 2    # cs (pair) columns
                    o0, o1 = xs // 4, xe // 4    # output columns
                    vector.wait_ge(s_xs[li], 16)
                    li += 1
                    if ct >= 2:
                        # cs WAW vs tile ct-2's final out; trivially satisfied
                        # by DVE program order, stated for the race checker.
                        vector.wait_ge(s_cmp, out_val[ct - 2][-1])
                    xv = xtile[:, xs:xe].rearrange("p (t two) -> p t two", two=2)
                    nc.vector.tensor_tensor_scan(
                        cs[:, c0:c1],
                        xv[:, :, 0],
                        xv[:, :, 1],
                        0.0,
                        mybir.AluOpType.add,
                        mybir.AluOpType.add,
                    ).then_inc(s_cmp, 1)
                    cval += 1
                    assert cval == scan_val[ct][p]
                    # scan -> out RAW on the same engine; for the checker.
                    vector.wait_ge(s_cmp, cval)
                    if p >= 2:
                        # carry column: total of pieces 0..p-1.
                        prev_end = pieces[ct][p - 1][1] // 2
                        if p == 2:
                            first_end = pieces[ct][0][1] // 2
                            nc.vector.tensor_add(
                                runc[:, ct:ct + 1],
                                cs[:, first_end - 1:first_end],
                                cs[:, prev_end - 1:prev_end],
                            ).then_inc(s_cmp, 1)
                        else:
                            nc.vector.tensor_add(
                                runc[:, ct:ct + 1],
                                runc[:, ct:ct + 1],
                                cs[:, prev_end - 1:prev_end],
                            ).then_inc(s_cmp, 1)
                        cval += 1
                        vector.wait_ge(s_cmp, cval)
                    if ct == 0 and p == 0:
                        # The recip table is only needed from the first OUT
                        # op on — never gate the scans on it.
                        vector.wait_ge(s_rt, n_banks)
                    csv = cs[:, c0:c1].rearrange("p (t two) -> p t two", two=2)
                    o_ap = ot[:, ct, o0:o1]
                    r_ap = rt[:, o0:o1]
                    if p == 0:
                        nc.vector.tensor_mul(
                            o_ap, csv[:, :, 1], r_ap
                        ).then_inc(s_cmp, 1)
                    elif p == 1:
                        # carry is just piece 0's total column.
                        nc.vector.scalar_tensor_tensor(
                            o_ap, csv[:, :, 1], cs[:, c0 - 1:c0], r_ap,
                            mybir.AluOpType.add, mybir.AluOpType.mult,
                        ).then_inc(s_cmp, 1)
                    else:
                        nc.vector.scalar_tensor_tensor(
                            o_ap, csv[:, :, 1], runc[:, ct:ct + 1], r_ap,
                            mybir.AluOpType.add, mybir.AluOpType.mult,
                        ).then_inc(s_cmp, 1)
                    cval += 1
                    assert cval == out_val[ct][p]

        @block.scalar
        def _(scalar):
            # Output stores on the ACT HWDGE ring; the PSUM->SBUF copies of
            # the broadcast recip table run on the idle ACT ALU.
            for k in range(n_banks):
                scalar.wait_ge(s_ps, k + 1)
                nc.scalar.copy(
                    rt[:, k * bank_cols:(k + 1) * bank_cols],
                    rps[:, k * bank_cols:(k + 1) * bank_cols],
                ).then_inc(s_rt, 1)
            n_stores = 0
            for ct in range(n_ct):
                for p, (xs, xe) in enumerate(pieces[ct]):
                    o0, o1 = xs // 4, xe // 4
                    scalar.wait_ge(s_cmp, out_val[ct][p])
                    scalar.dma_start(
                        out=outT[ct * P:(ct + 1) * P, o0:o1],
                        in_=ot[:, ct, o0:o1],
                    ).then_inc(s_out, 16)
                    n_stores += 1
            # Outputs must be in HBM before the kernel exits.
            scalar.wait_ge(s_out, 16 * n_stores)

    return nc


def _recip_row(out_len):
    r = 1.0 / (SF * np.arange(1, out_len + 1, dtype=np.float64))
    return r.astype(np.float32).reshape(1, out_len)


def kernel(x: np.ndarray) -> np.ndarray:
    b, length, d = x.shape
    out_len = length // SF
    # One batch per core, channels on partitions: host-side transpose is
    # pure layout so every DMA in the kernel is contiguous.
    xT = np.ascontiguousarray(np.swapaxes(np.asarray(x, dtype=np.float32), 1, 2))
    recip = _recip_row(out_len)
    in_maps = [{"xT": xT[i], "recip": recip} for i in range(b)]
    nc = build_bass(d=d, length=length)
    res = run_bass_kernel_spmd(nc, in_maps, core_ids=list(range(b)))
    outT = np.stack([res.results[i]["outT"] for i in range(b)])
    return np.ascontiguousarray(np.swapaxes(outT, 1, 2))


# revision 28
# speedup vs baseline: 1.0627x; 1.0069x over previous
"""Trainium2 Bass kernel for causal average pooling (downsampling).

Reference op: out[b, i, d] = mean(x[b, :(i+1)*4, d]) over the time axis,
for x of shape (8, 8192, 512) f32 -> out (8, 2048, 512) f32.

Strategy
--------
Data-parallel over batch: one batch per NeuronCore (8 cores), no
cross-core communication.

Per core the math is, for each channel d independently, a prefix sum
over time sampled every SF=4 steps, scaled by 1/(4(i+1)).  We lay the
data out as [channel partitions, time free-dim] (the host pre-transposes
each batch, which is pure layout) and use the hardware prefix scan
`tensor_tensor_scan` on the vector engine:

    state = (data0[t] + state) + data1[t]

Feeding data0 = x[:, 0::2] and data1 = x[:, 1::2] gives the cumulative
sum over PAIRS: cs2[:, j] = sum(x[:, :2j+2]).  Output i of the reference
needs sum(x[:, :4i+4]) = cs2[:, 2i+1]: a strided gather of the odd
columns times a 1/(4(i+1)) table (an 8 KB host row, partition-broadcast
on GpSimd so it costs no DMA-fabric bandwidth).

Each 128-channel tile's time axis is cut into pieces which are scanned
INDEPENDENTLY (initial=0.0 — chaining through an AP initial measured
~2.3 us slower per scan).  A piece's missing carry (total of the earlier
pieces, maintained as a [128,1] running column) is folded into its
output op for free with scalar_tensor_tensor:
    out = (cs_local + carry) * recip.
The kernel is DMA-fabric-bound (~430 GB/s shared by loads+stores), so
the last tile is tapered into small pieces to shrink the serial tail
(last-load receipt -> scan -> out -> store -> receipt).

Pipeline per core (xT [512 chan, 8192 time], 4 channel tiles):
  SP ring:   x piece loads (2 MiB mid-stream, tapered at the end)
  ACT ring:  recip row load (8 KB), then per-piece output stores
  GpSimd:    recip partition-broadcast (once)
  DVE:       per piece: scan + gather*recip (TT / STT), carry columns

Written in raw Bass (not Tile): the walrus build in this container
enforces at most ONE semaphore wait per hardware instruction, so all
cross-engine waits are standalone wait_ge ops.  Each load gets its own
semaphore because completions of back-to-back DMAs on one HWDGE ring
are unordered.
"""

import sys

if "/opt/trn_rl_repo" not in sys.path:
    sys.path.insert(0, "/opt/trn_rl_repo")

import numpy as np

import concourse.bass as bass
import concourse.mybir as mybir
from concourse.bass_utils import run_bass_kernel_spmd

P = 128           # SBUF partitions
SF = 4            # pooling factor
B, L, D = 8, 8192, 512
N_CORES = 8


def _pieces(n_ct, length):
    """Per-tile piece boundaries in x columns. Mid-stream pieces are half a
    tile (2 MiB); the first tile ramps up in small pieces (first scan starts
    after 0.5 MiB instead of 2 MiB) and the final tile tapers down so the
    serial tail after the last load is short."""
    halves = [(0, length // 2), (length // 2, length)]
    if length < 4096:
        return [halves] * n_ct
    ramp = halves
    taper = [
        (0, length // 2),
        (length // 2, 3 * length // 4),
        (3 * length // 4, 7 * length // 8),
        (7 * length // 8, length),
    ]
    return [ramp] + [halves] * (n_ct - 2) + [taper]


def build_bass(d=D, length=L):
    half = length // 2          # scan steps per tile (pairs)
    out_len = length // SF
    n_ct = d // P
    assert d % P == 0 and length % (2 * SF * 8) == 0

    nc = bass.Bass()
    xT = nc.dram_tensor("xT", [d, length], mybir.dt.float32, kind="ExternalInput")
    recip = nc.dram_tensor(
        "recip", [1, out_len], mybir.dt.float32, kind="ExternalInput"
    )
    outT = nc.dram_tensor(
        "outT", [d, out_len], mybir.dt.float32, kind="ExternalOutput"
    )

    pieces = _pieces(n_ct, length)
    n_loads = sum(len(p) for p in pieces)

    # DVE op index bookkeeping (s_cmp is incremented by every DVE op).
    cmp_val = 0
    scan_val = [[None] * len(pieces[ct]) for ct in range(n_ct)]  # scan done
    out_val = [[None] * len(pieces[ct]) for ct in range(n_ct)]   # out op done

    with (
        nc.sbuf_tensor([P, length], mybir.dt.float32) as xt0,
        nc.sbuf_tensor([P, length], mybir.dt.float32) as xt1,
        nc.sbuf_tensor([P, length], mybir.dt.float32) as xt2,
        nc.sbuf_tensor([P, half], mybir.dt.float32) as cs0,
        nc.sbuf_tensor([P, half], mybir.dt.float32) as cs1,
        nc.sbuf_tensor([1, out_len], mybir.dt.float32) as rrow,
        nc.sbuf_tensor([1, P], mybir.dt.float32) as ones,
        nc.psum_tensor([P, out_len], mybir.dt.float32) as rps,
        nc.sbuf_tensor([P, out_len], mybir.dt.float32) as rt,
        nc.sbuf_tensor([P, n_ct], mybir.dt.float32) as runc,
        nc.sbuf_tensor([P, n_ct, out_len], mybir.dt.float32) as ot,
        nc.semaphore("s_rrow") as s_rrow,
        nc.semaphore("s_ones") as s_ones,
        nc.semaphore("s_ps") as s_ps,
        nc.semaphore("s_rt") as s_rt,
        nc.semaphore("s_cmp") as s_cmp,
        nc.semaphore("s_out") as s_out,
        nc.Block() as block,
    ):
        n_banks = (out_len + 511) // 512
        bank_cols = min(512, out_len)
        s_xs = [nc.alloc_semaphore(f"s_x{i}") for i in range(n_loads)]
        xts = [xt0, xt1, xt2]
        n_xb = len(xts)
        css = [cs0, cs1]

        # ---- plan the DVE op order so cross-engine wait values are known ---
        # per tile: [scan p, (run-col update?), out p] for each piece p
        for ct in range(n_ct):
            for p in range(len(pieces[ct])):
                cmp_val += 1                     # scan
                scan_val[ct][p] = cmp_val
                if p >= 2:
                    cmp_val += 1                 # running-carry column update
                cmp_val += 1                     # out op
                out_val[ct][p] = cmp_val

        @block.sync
        def _(sync):
            # 8 KB recip row first (negligible bytes, needed by the PE
            # broadcast early), then x loads, all on the SP HWDGE ring.
            sync.dma_start(out=rrow[:, :], in_=recip[:, :]).then_inc(s_rrow, 16)
            li = 0
            for ct in range(n_ct):
                for p, (xs, xe) in enumerate(pieces[ct]):
                    if ct >= n_xb:
                        # buffer WAR: last scan of tile ct-n_xb whose region
                        # overlaps this piece must be done with the buffer.
                        last = max(
                            pp for pp, (ps, pe) in enumerate(pieces[ct - n_xb])
                            if ps < xe and pe > xs
                        )
                        sync.wait_ge(s_cmp, scan_val[ct - n_xb][last])
                    sync.dma_start(
                        out=xts[ct % n_xb][:, xs:xe],
                        in_=xT[ct * P:(ct + 1) * P, xs:xe],
                    ).then_inc(s_xs[li], 16)
                    li += 1

        @block.gpsimd
        def _(gpsimd):
            nc.gpsimd.memset(ones[:, :], 1.0).then_inc(s_ones, 1)

        @block.tensor
        def _(tensor):
            # Broadcast the 8 KB recip row to all 128 partitions on the
            # (otherwise idle) PE: ones[1,128].T @ rrow[1,bank] replicates the
            # row into PSUM, so the table never crosses the DMA fabric at
            # full size.  One matmul per PSUM bank (N<=512).
            tensor.wait_ge(s_rrow, 16)
            tensor.wait_ge(s_ones, 1)
            for k in range(n_banks):
                nc.tensor.matmul(
                    rps[:, k * bank_cols:(k + 1) * bank_cols],
                    ones[:, :],
                    rrow[:, k * bank_cols:(k + 1) * bank_cols],
                    start=True,
                    stop=True,
                ).then_inc(s_ps, 1)

        @block.vector
        def _(vector):
            li = 0
            cval = 0
            for ct in range(n_ct):
                cs = css[ct % 2][:, :]
                xtile = xts[ct % n_xb]
                for p, (xs, xe) in enumerate(pieces[ct]):
                    c0, c1 = xs // 2, xe // 2    # cs (pair) columns
                    o0, o1 = xs // 4, xe // 4    # output columns
                    vector.wait_ge(s_xs[li], 16)
                    li += 1
                    if ct >= 2:
                        # cs WAW vs tile ct-2's final out; trivially satisfied
                        # by DVE program order, stated for the race checker.
                        vector.wait_ge(s_cmp, out_val[ct - 2][-1])
                    xv = xtile[:, xs:xe].rearrange("p (t two) -> p t two", two=2)
                    nc.vector.tensor_tensor_scan(
                        cs[:, c0:c1],
                        xv[:, :, 0],
                        xv[:, :, 1],
                        0.0,
                        mybir.AluOpType.add,
                        mybir.AluOpType.add,
                    ).then_inc(s_cmp, 1)
                    cval += 1
                    assert cval == scan_val[ct][p]
                    # scan -> out RAW on the same engine; for the checker.
                    vector.wait_ge(s_cmp, cval)
                    if p >= 2:
                        # carry column: total of pieces 0..p-1.
                        prev_end = pieces[ct][p - 1][1] // 2
                        if p == 2:
                            first_end = pieces[ct][0][1] // 2
                            nc.vector.tensor_add(
                                runc[:, ct:ct + 1],
                                cs[:, first_end - 1:first_end],
                                cs[:, prev_end - 1:prev_end],
                            ).then_inc(s_cmp, 1)
                        else:
                            nc.vector.tensor_add(
                                runc[:, ct:ct + 1],
                                runc[:, ct:ct + 1],
                                cs[:, prev_end - 1:prev_end],
                            ).then_inc(s_cmp, 1)
                        cval += 1
                        vector.wait_ge(s_cmp, cval)
                    if ct == 0 and p == 0:
                        # The recip table is only needed from the first OUT
                        # op on — never gate the scans on it.
                        vector.wait_ge(s_rt, n_banks)
                    csv = cs[:, c0:c1].rearrange("p (t two) -> p t two", two=2)
                    o_ap = ot[:, ct, o0:o1]
                    r_ap = rt[:, o0:o1]
                    if p == 0:
                        nc.vector.tensor_mul(
                            o_ap, csv[:, :, 1], r_ap
                        ).then_inc(s_cmp, 1)
                    elif p == 1:
                        # carry is just piece 0's total column.
                        nc.vector.scalar_tensor_tensor(
                            o_ap, csv[:, :, 1], cs[:, c0 - 1:c0], r_ap,
                            mybir.AluOpType.add, mybir.AluOpType.mult,
                        ).then_inc(s_cmp, 1)
                    else:
                        nc.vector.scalar_tensor_tensor(
                            o_ap, csv[:, :, 1], runc[:, ct:ct + 1], r_ap,
                            mybir.AluOpType.add, mybir.AluOpType.mult,
                        ).then_inc(s_cmp, 1)
                    cval += 1
                    assert cval == out_val[ct][p]

        @block.scalar
        def _(scalar):
            # Output stores on the ACT HWDGE ring; the PSUM->SBUF copies of
            # the broadcast recip table run on the idle ACT ALU.
            for k in range(n_banks):
                scalar.wait_ge(s_ps, k + 1)
                nc.scalar.copy(
                    rt[:, k * bank_cols:(k + 1) * bank_cols],
                    rps[:, k * bank_cols:(k + 1) * bank_cols],
                ).then_inc(s_rt, 1)
            n_stores = 0
            for ct in range(n_ct):
                for p, (xs, xe) in enumerate(pieces[ct]):
                    o0, o1 = xs // 4, xe // 4
                    scalar.wait_ge(s_cmp, out_val[ct][p])
                    scalar.dma_start(
                        out=outT[ct * P:(ct + 1) * P, o0:o1],
                        in_=ot[:, ct, o0:o1],
                    ).then_inc(s_out, 16)
                    n_stores += 1
            # Outputs must be in HBM before the kernel exits.
            scalar.wait_ge(s_out, 16 * n_stores)

    return nc


def _recip_row(out_len):
    r = 1.0 / (SF * np.arange(1, out_len + 1, dtype=np.float64))
    return r.astype(np.float32).reshape(1, out_len)


def kernel(x: np.ndarray) -> np.ndarray:
    b, length, d = x.shape
    out_len = length // SF
    # One batch per core, channels on partitions: host-side transpose is
    # pure layout so every DMA in the kernel is contiguous.
    xT = np.ascontiguousarray(np.swapaxes(np.asarray(x, dtype=np.float32), 1, 2))
    recip = _recip_row(out_len)
    in_maps = [{"xT": xT[i], "recip": recip} for i in range(b)]
    nc = build_bass(d=d, length=length)
    res = run_bass_kernel_spmd(nc, in_maps, core_ids=list(range(b)))
    outT = np.stack([res.results[i]["outT"] for i in range(b)])
    return np.ascontiguousarray(np.swapaxes(outT, 1, 2))
